# revision 6
# baseline (speedup 1.0000x reference)
"""Trainium2 Bass kernel for nn_ALAttention (sparse local attention).

Strategy: shard the 64 image rows across 8 cores (8 query rows each). All 33
attention targets of a query in row r lie within image rows r-4..r+4, so each
core only needs a 16-row halo slab of x. Inside a core everything is dense
matmul: QKV 1x1-conv GEMM, masked dense local attention over the slab's 1024
key positions (mask prebuilt on host from attn_idx), proj GEMM. No
inter-core communication.

Graph is SPMD-identical across cores; all per-core differences are carried by
the input data (slab slice, mask). Borders are handled by virtually centering
each slab (zero-padding x above row 0 / below row 63); padded keys are never
referenced by attn_idx so the mask kills them.
"""
import os
import sys
import types

sys.path.insert(0, "/opt/trn_rl_repo")

import numpy as np
import ml_dtypes

from concourse import bacc, tile, mybir
from concourse import bass_utils
from concourse import masks as bass_masks
from concourse.bass_utils import run_bass_kernel_spmd

F32 = mybir.dt.float32
BF16 = mybir.dt.bfloat16
AF = mybir.ActivationFunctionType

B = 2
C = 384
HH = WW = 64
HEADS = 6
HD = 64          # head dim
NCORES = 8
ROWS = 8         # query rows per core
SLAB = 16        # halo slab rows per core
SCOLS = SLAB * WW      # 1024 slab key positions
QCOLS = ROWS * WW      # 512 queries per core
NKC = SCOLS // 128     # 8 key chunks
SCALE = float(HD) ** -0.5

# exec info stash (test.py reads these)
LAST_EXEC_NS = None
LAST_TRACE = None

_NC_CACHE = {}


def _register_ntff_hook():
    """The image's antenv lacks axon_hooks; register the NTFF profile hook
    the same way trn_agent_boot would."""
    if "antenv.axon_hooks" in sys.modules:
        return
    try:
        from trn_agent_boot.trn_boot import _ntff_profile_via_ctypes
        hook = _ntff_profile_via_ctypes("/opt/axon/libaxon_pjrt.so")
    except Exception:
        hook = None
    mod = types.ModuleType("antenv.axon_hooks")
    mod.get_axon_ntff_profile_hook = lambda: hook
    mod.set_axon_ntff_profile_hook = lambda h: None
    sys.modules["antenv.axon_hooks"] = mod
    bass_utils.upload_artifacts = lambda tmpdir: "local://skipped"


def build_graph():
    nc = bacc.Bacc("TRN2", target_bir_lowering=False, debug=False,
                   num_devices=NCORES)

    xs_e = nc.dram_tensor("xs", [B, C, SCOLS], BF16, kind="ExternalInput").ap()
    wqkvT_e = nc.dram_tensor("wqkvT", [C, 3 * C], BF16, kind="ExternalInput").ap()
    bqkv_e = nc.dram_tensor("bqkv", [128, 9], F32, kind="ExternalInput").ap()
    wprojT_e = nc.dram_tensor("wprojT", [C, C], BF16, kind="ExternalInput").ap()
    bproj_e = nc.dram_tensor("bproj", [128, 3], F32, kind="ExternalInput").ap()
    mask_e = nc.dram_tensor("mask", [NKC, 128, QCOLS], BF16,
                            kind="ExternalInput").ap()
    out_e = nc.dram_tensor("out", [B, C, QCOLS], F32, kind="ExternalOutput").ap()

    with tile.TileContext(nc) as tc:
        with (
            tc.tile_pool(name="const", bufs=1) as cpool,
            tc.tile_pool(name="xin", bufs=2) as xpool,
            tc.tile_pool(name="qkv", bufs=2) as qkvpool,
            tc.tile_pool(name="vt", bufs=2) as vtpool,
            tc.tile_pool(name="esb", bufs=3) as epool,
            tc.tile_pool(name="osb", bufs=2) as opool,
            tc.tile_pool(name="sc", bufs=3) as scpool,
            tc.tile_pool(name="psA", bufs=3, space="PSUM") as psA,
            tc.tile_pool(name="psV", bufs=2, space="PSUM") as psV,
            tc.tile_pool(name="psO", bufs=2, space="PSUM") as psO,
        ):
            # ---- constants / weights ----
            w_sb = cpool.tile([128, 3, 3 * C], BF16, tag="wqkv")  # [kchunk]
            for k in range(3):
                nc.sync.dma_start(w_sb[:, k, :], wqkvT_e[128 * k:128 * (k + 1), :])
            wp_sb = cpool.tile([128, 3, C], BF16, tag="wproj")
            for k in range(3):
                nc.sync.dma_start(wp_sb[:, k, :], wprojT_e[128 * k:128 * (k + 1), :])
            bq_sb = cpool.tile([128, 9], F32, tag="bqkv")
            nc.sync.dma_start(bq_sb[:], bqkv_e[:])
            bp_sb = cpool.tile([128, 3], F32, tag="bproj")
            nc.sync.dma_start(bp_sb[:], bproj_e[:])
            ident = cpool.tile([128, 128], BF16, tag="ident")
            bass_masks.make_identity(nc, ident[:])
            mask_sb = cpool.tile([128, NKC, QCOLS], BF16, tag="mask")
            for j in range(NKC):
                nc.sync.dma_start(mask_sb[:, j, :], mask_e[j])

            for b in range(B):
                # ---- load x slab ----
                x_sb = xpool.tile([128, 3, SCOLS], BF16)
                for k in range(3):
                    nc.sync.dma_start(x_sb[:, k, :],
                                      xs_e[b, 128 * k:128 * (k + 1), :])

                # ---- QKV GEMM: qkv[m][128, 1024] = wqkvT.T @ x + b ----
                qkv_sb = qkvpool.tile([128, 9, SCOLS], BF16)
                for m in range(9):
                    is_q = m < 3
                    # q only needs the core's own 512 columns
                    nlist = [(256, 768)] if is_q else [(0, 512), (512, 1024)]
                    for (n0, n1) in nlist:
                        ps = psA.tile([128, 512], F32, tag="mm")
                        for k in range(3):
                            nc.tensor.matmul(
                                ps[:, :n1 - n0],
                                w_sb[:, k, 128 * m:128 * (m + 1)],
                                x_sb[:, k, n0:n1],
                                start=(k == 0), stop=(k == 2),
                            )
                        nc.scalar.activation(
                            qkv_sb[:, m, n0:n1], ps[:, :n1 - n0],
                            AF.Identity,
                            bias=bq_sb[:, m:m + 1],
                            scale=SCALE if is_q else 1.0,
                        )

                # ---- attention per head ----
                ocat = [opool.tile([128, QCOLS], BF16, tag=f"ocat{c}",
                                   name=f"ocat{c}")
                        for c in range(3)]
                for h in range(HEADS):
                    mc, mo = h // 2, 64 * (h % 2)
                    q_ap = qkv_sb[mo:mo + 64, mc, 256:768]
                    k_mc = 3 + mc
                    v_mc = 6 + mc

                    # transpose V: [64, 1024] -> keys-major V_aug [128, 8, 65]
                    v_sb = vtpool.tile([128, NKC, 65], BF16)
                    for j in range(NKC):
                        vt_ps = psV.tile([128, 64], BF16, tag="vt")
                        nc.tensor.transpose(
                            vt_ps[:],
                            qkv_sb[mo:mo + 64, v_mc, 128 * j:128 * (j + 1)],
                            ident[mo:mo + 64, mo:mo + 64],
                        )
                        nc.scalar.copy(v_sb[:, j, 0:64], vt_ps[:])
                        nc.gpsimd.memset(v_sb[:, j, 64:65], 1.0)

                    # S^T chunks + exp + mask; O^T accumulate
                    ot = psO.tile([65, QCOLS], F32, tag="ot")
                    for j in range(NKC):
                        st = psA.tile([128, 512], F32, tag="mm")
                        nc.tensor.matmul(
                            st[:],
                            qkv_sb[mo:mo + 64, k_mc, 128 * j:128 * (j + 1)],
                            q_ap,
                            start=True, stop=True,
                        )
                        e_sb = epool.tile([128, QCOLS], BF16)
                        nc.scalar.activation(e_sb[:], st[:], AF.Exp)
                        nc.vector.tensor_tensor(
                            e_sb[:], e_sb[:], mask_sb[:, j, :],
                            mybir.AluOpType.mult)
                        nc.tensor.matmul(
                            ot[:],
                            v_sb[:, j, :],
                            e_sb[:],
                            start=(j == 0), stop=(j == NKC - 1),
                            skip_group_check=True,
                        )

                    # normalize: O[0:64] * (1/rowsum) -> ocat
                    rrow = scpool.tile([1, QCOLS], F32, tag="rrow")
                    nc.vector.reciprocal(rrow[:], ot[64:65, :])
                    rb = scpool.tile([64, QCOLS], F32, tag="rb")
                    nc.gpsimd.partition_broadcast(rb[:], rrow[:])
                    nc.vector.tensor_tensor(
                        ocat[mc][mo:mo + 64, :], ot[0:64, :], rb[:],
                        mybir.AluOpType.mult)

                # ---- proj GEMM + bias ----
                for m in range(3):
                    ps = psA.tile([128, 512], F32, tag="mm")
                    for k in range(3):
                        nc.tensor.matmul(
                            ps[:],
                            wp_sb[:, k, 128 * m:128 * (m + 1)],
                            ocat[k][:],
                            start=(k == 0), stop=(k == 2),
                        )
                    o_sb = scpool.tile([128, QCOLS], F32, tag="out")
                    nc.scalar.activation(
                        o_sb[:], ps[:], AF.Identity,
                        bias=bp_sb[:, m:m + 1], scale=1.0)
                    nc.sync.dma_start(out_e[b, 128 * m:128 * (m + 1), :],
                                      o_sb[:])

    nc.compile()
    return nc


def _build_inputs(x, w_qkv, b_qkv, w_proj, b_proj, attn_idx):
    bf = ml_dtypes.bfloat16
    x = np.asarray(x, np.float32)
    attn_idx = np.asarray(attn_idx)

    xp = np.zeros((B, C, HH + 8, WW), np.float32)
    xp[:, :, 4:4 + HH, :] = x
    xp = xp.astype(bf)

    wqkvT = np.ascontiguousarray(np.asarray(w_qkv, np.float32).T).astype(bf)
    wprojT = np.ascontiguousarray(np.asarray(w_proj, np.float32).T).astype(bf)

    b_adj = np.asarray(b_qkv, np.float32).copy()
    b_adj[:C] *= SCALE
    bqkv = np.ascontiguousarray(b_adj.reshape(9, 128).T)  # [128, 9]
    bproj = np.ascontiguousarray(
        np.asarray(b_proj, np.float32).reshape(3, 128).T)  # [128, 3]

    in_maps = []
    for i in range(NCORES):
        slab = np.ascontiguousarray(
            xp[:, :, 8 * i:8 * i + SLAB, :]).reshape(B, C, SCOLS)
        q0 = 8 * i * WW  # first query index (row 8i, col 0)
        gq = np.arange(q0, q0 + QCOLS)
        aidx = attn_idx[gq].astype(np.int64)          # [512, 33] global keys
        local = aidx - (8 * i - 4) * WW               # slab-local positions
        assert local.min() >= 0 and local.max() < SCOLS, \
            f"core {i}: attn target outside slab"
        m = np.zeros((NKC, 128, QCOLS), np.float32)
        qq = np.repeat(np.arange(QCOLS), aidx.shape[1])
        ll = local.ravel()
        m[ll // 128, ll % 128, qq] = 1.0
        in_maps.append({
            "xs": slab,
            "wqkvT": wqkvT,
            "bqkv": bqkv,
            "wprojT": wprojT,
            "bproj": bproj,
            "mask": m.astype(bf),
        })
    return in_maps


def kernel(x, w_qkv, b_qkv, w_proj, b_proj, attn_idx):
    global LAST_EXEC_NS, LAST_TRACE
    _register_ntff_hook()
    if "graph" not in _NC_CACHE:
        _NC_CACHE["graph"] = build_graph()
    nc = _NC_CACHE["graph"]
    in_maps = _build_inputs(x, w_qkv, b_qkv, w_proj, b_proj, attn_idx)
    trace = bool(int(os.environ.get("BASSK_TRACE", "0")))
    res = run_bass_kernel_spmd(nc, in_maps, core_ids=list(range(NCORES)),
                               trace=trace)
    LAST_EXEC_NS = res.exec_time_ns
    if res.instructions_and_trace is not None:
        LAST_TRACE = res.instructions_and_trace[1]
    out = np.empty((B, C, HH, WW), np.float32)
    for i in range(NCORES):
        o = res.results[i]["out"].reshape(B, C, ROWS, WW)
        out[:, :, 8 * i:8 * i + ROWS, :] = o
    return out


# revision 14
# speedup vs baseline: 1.5776x; 1.5776x over previous
"""Trainium2 Bass kernel for nn_ALAttention (sparse local attention).

Sharding: 64 image rows split across 8 cores (8 query rows each). All 33
attention targets of a query in row r lie within rows r-4..r+4, so each core
works on a host-sliced 16-row halo slab of x (virtually centered, zero-padded
at borders -> identical SPMD graph; padded keys are masked out). Per core:
QKV GEMM (bf16, fused bias, q-scale folded into host-prescaled bias), masked
dense local attention in S^T=[keys,q] layout (host-built mask from attn_idx,
exp -> mask-mult -> V_aug matmul whose ones-column yields the softmax
denominator), normalize via fast-reciprocal + partition-broadcast, proj GEMM
with per-partition bias. No inter-core communication.
"""
import os
import sys
import types

sys.path.insert(0, "/opt/trn_rl_repo")

import numpy as np
import ml_dtypes

from concourse import bacc, tile, mybir
from concourse import bass_utils
from concourse.bass_utils import run_bass_kernel_spmd

F32 = mybir.dt.float32
BF16 = mybir.dt.bfloat16
AF = mybir.ActivationFunctionType

B = 2
C = 384
HH = WW = 64
HEADS = 6
NCORES = 8
ROWS = 8               # query rows per core
SLAB = 16              # halo slab rows per core
SCOLS = SLAB * WW      # 1024 slab key positions
QCOLS = ROWS * WW      # 512 queries per core
NKC = SCOLS // 128     # 8 key chunks
SCALE = float(64) ** -0.5

LAST_EXEC_NS = None
LAST_TRACE = None
_NC_CACHE = {}


def _register_ntff_hook():
    if "antenv.axon_hooks" in sys.modules:
        return
    try:
        from trn_agent_boot.trn_boot import _ntff_profile_via_ctypes
        hook = _ntff_profile_via_ctypes("/opt/axon/libaxon_pjrt.so")
    except Exception:
        hook = None
    mod = types.ModuleType("antenv.axon_hooks")
    mod.get_axon_ntff_profile_hook = lambda: hook
    mod.set_axon_ntff_profile_hook = lambda h: None
    sys.modules["antenv.axon_hooks"] = mod
    bass_utils.upload_artifacts = lambda tmpdir: "local://skipped"


def build_graph():
    nc = bacc.Bacc("TRN2", target_bir_lowering=False, debug=False,
                   num_devices=NCORES)

    xs_e = nc.dram_tensor("xs", [B, C, SCOLS], BF16, kind="ExternalInput").ap()
    wqkvT_e = nc.dram_tensor("wqkvT", [C, 3 * C], BF16, kind="ExternalInput").ap()
    bqkv_e = nc.dram_tensor("bqkv", [128, 9], F32, kind="ExternalInput").ap()
    wprojT_e = nc.dram_tensor("wprojT", [C, C], BF16, kind="ExternalInput").ap()
    bproj_e = nc.dram_tensor("bproj", [128, 3], F32, kind="ExternalInput").ap()
    mask_e = nc.dram_tensor("mask", [4, 128, 1024], BF16,
                            kind="ExternalInput").ap()
    out_e = nc.dram_tensor("out", [B, C, QCOLS], F32, kind="ExternalOutput").ap()

    with tile.TileContext(nc) as tc:
        with (
            tc.tile_pool(name="const", bufs=1) as cpool,
            tc.tile_pool(name="xin", bufs=2) as xpool,
            tc.tile_pool(name="qkv", bufs=2) as qkvpool,
            tc.tile_pool(name="vt", bufs=2) as vtpool,
            tc.tile_pool(name="esb", bufs=3) as epool,
            tc.tile_pool(name="osb", bufs=2) as opool,
            tc.tile_pool(name="sc", bufs=3) as scpool,
            tc.tile_pool(name="psA", bufs=2, space="PSUM") as psA,
            tc.tile_pool(name="psO", bufs=2, space="PSUM") as psO,
            tc.tile_pool(name="psV", bufs=2, space="PSUM") as psV,
        ):
            # ---- constants / weights ----
            w_sb = cpool.tile([128, 3, 3 * C], BF16, tag="wqkv")
            for k in range(3):
                nc.sync.dma_start(w_sb[:, k, :], wqkvT_e[128 * k:128 * (k + 1), :])
            wp_sb = cpool.tile([128, 3, C], BF16, tag="wproj")
            for k in range(3):
                nc.sync.dma_start(wp_sb[:, k, :], wprojT_e[128 * k:128 * (k + 1), :])
            bq_sb = cpool.tile([128, 9], F32, tag="bqkv")
            nc.sync.dma_start(bq_sb[:], bqkv_e[:])
            bp_sb = cpool.tile([128, 3], F32, tag="bproj")
            nc.sync.dma_start(bp_sb[:], bproj_e[:])
            ident = cpool.tile([128, 128], BF16, tag="ident")
            from concourse import masks as bass_masks
            bass_masks.make_identity(nc, ident[:])
            mask_sb = cpool.tile([128, 4 * 1024], BF16, tag="mask")
            for jp in range(4):
                nc.sync.dma_start(mask_sb[:, 1024 * jp:1024 * (jp + 1)],
                                  mask_e[jp])

            for b in range(B):
                # ---- load x slab ----
                x_sb = xpool.tile([128, 3, SCOLS], BF16)
                for k in range(3):
                    nc.sync.dma_start(x_sb[:, k, :],
                                      xs_e[b, 128 * k:128 * (k + 1), :])

                # ---- QKV GEMM ----
                qkv_sb = qkvpool.tile([128, 9, SCOLS], BF16)
                for m in range(9):
                    is_q = m < 3
                    ps = psA.tile([128, 1024], F32, tag="mm")
                    if is_q:
                        for k in range(3):
                            nc.tensor.matmul(
                                ps[:, 0:512],
                                w_sb[:, k, 128 * m:128 * (m + 1)],
                                x_sb[:, k, 256:768],
                                start=(k == 0), stop=(k == 2))
                        nc.scalar.activation(
                            qkv_sb[:, m, 256:768], ps[:, 0:512],
                            AF.Identity, bias=bq_sb[:, m:m + 1], scale=SCALE)
                    else:
                        for n in range(2):
                            for k in range(3):
                                nc.tensor.matmul(
                                    ps[:, 512 * n:512 * (n + 1)],
                                    w_sb[:, k, 128 * m:128 * (m + 1)],
                                    x_sb[:, k, 512 * n:512 * (n + 1)],
                                    start=(k == 0), stop=(k == 2))
                        nc.scalar.activation(
                            qkv_sb[:, m, :], ps[:],
                            AF.Identity, bias=bq_sb[:, m:m + 1], scale=1.0)

                # ---- attention per head ----
                ocat = [opool.tile([128, QCOLS], BF16, tag=f"ocat{c}",
                                   name=f"ocat{c}") for c in range(3)]
                for h in range(HEADS):
                    mc, mo = h // 2, 64 * (h % 2)
                    q_ap = qkv_sb[mo:mo + 64, mc, 256:768]
                    k_mc = 3 + mc
                    v_mc = 6 + mc

                    # V -> keys-major: PE transposes into one wide psum tile,
                    # then a single strided copy into 128-wide slots
                    v_ps = psV.tile([128, NKC, 64], BF16, tag="vt")
                    for j in range(NKC):
                        nc.tensor.transpose(
                            v_ps[:, j, :],
                            qkv_sb[mo:mo + 64, v_mc, 128 * j:128 * (j + 1)],
                            ident[mo:mo + 64, mo:mo + 64])
                    v_sb = vtpool.tile([128, NKC, 128], BF16)
                    nc.scalar.copy(v_sb[:, :, 0:64], v_ps[:])
                    nc.gpsimd.memset(v_sb[:, :, 64:65], 1.0)

                    ot = psO.tile([128, QCOLS], F32, tag="ot")
                    for jp in range(4):
                        st = psA.tile([128, 1024], F32, tag="mm")
                        for jj in range(2):
                            j = 2 * jp + jj
                            nc.tensor.matmul(
                                st[:, 512 * jj:512 * (jj + 1)],
                                qkv_sb[mo:mo + 64, k_mc, 128 * j:128 * (j + 1)],
                                q_ap, start=True, stop=True)
                        e_sb = epool.tile([128, 1024], BF16)
                        nc.scalar.activation(e_sb[:], st[:], AF.Exp)
                        nc.vector.tensor_tensor(
                            e_sb[:], e_sb[:],
                            mask_sb[:, 1024 * jp:1024 * (jp + 1)],
                            mybir.AluOpType.mult)
                        for jj in range(2):
                            j = 2 * jp + jj
                            nc.tensor.matmul(
                                ot[:], v_sb[:, j, :],
                                e_sb[:, 512 * jj:512 * (jj + 1)],
                                start=(j == 0), stop=(j == NKC - 1),
                                skip_group_check=True)

                    # normalize V-rows 0:64 by 1/rowsum (row 64 of ot)
                    srow = scpool.tile([1, QCOLS], F32, tag="srow")
                    nc.vector.tensor_copy(srow[:], ot[64:65, :])
                    rrow = scpool.tile([1, QCOLS], F32, tag="rrow")
                    nc.vector.reciprocal_approx_fast(rrow[:], srow[:])
                    rb = scpool.tile([64, QCOLS], F32, tag="rb")
                    nc.gpsimd.partition_broadcast(rb[:], rrow[:])
                    nc.vector.tensor_tensor(
                        ocat[mc][mo:mo + 64, :], ot[0:64, :], rb[:],
                        mybir.AluOpType.mult)

                # ---- proj GEMM + bias ----
                for m in range(3):
                    ps = psA.tile([128, 1024], F32, tag="mm")
                    for k in range(3):
                        nc.tensor.matmul(
                            ps[:, 0:512], wp_sb[:, k, 128 * m:128 * (m + 1)],
                            ocat[k][:], start=(k == 0), stop=(k == 2))
                    o_sb = scpool.tile([128, QCOLS], F32, tag="out")
                    nc.scalar.activation(
                        o_sb[:], ps[:, 0:512], AF.Identity,
                        bias=bp_sb[:, m:m + 1], scale=1.0)
                    nc.sync.dma_start(out_e[b, 128 * m:128 * (m + 1), :],
                                      o_sb[:])

    nc.compile()
    return nc


def _build_inputs(x, w_qkv, b_qkv, w_proj, b_proj, attn_idx):
    bf = ml_dtypes.bfloat16
    x = np.asarray(x, np.float32)
    attn_idx = np.asarray(attn_idx)

    xp = np.zeros((B, C, HH + 8, WW), np.float32)
    xp[:, :, 4:4 + HH, :] = x
    xp = xp.astype(bf)

    wqkvT = np.ascontiguousarray(np.asarray(w_qkv, np.float32).T).astype(bf)
    wprojT = np.ascontiguousarray(np.asarray(w_proj, np.float32).T).astype(bf)

    b_adj = np.asarray(b_qkv, np.float32).copy()
    b_adj[:C] *= SCALE
    bqkv = np.ascontiguousarray(b_adj.reshape(9, 128).T)
    bproj = np.ascontiguousarray(
        np.asarray(b_proj, np.float32).reshape(3, 128).T)

    in_maps = []
    for i in range(NCORES):
        slab = np.ascontiguousarray(
            xp[:, :, 8 * i:8 * i + SLAB, :]).reshape(B, C, SCOLS)
        q0 = 8 * i * WW
        gq = np.arange(q0, q0 + QCOLS)
        aidx = attn_idx[gq].astype(np.int64)
        local = aidx - (8 * i - 4) * WW
        assert local.min() >= 0 and local.max() < SCOLS, \
            f"core {i}: attn target outside slab"
        m = np.zeros((NKC, 128, QCOLS), np.float32)
        qq = np.repeat(np.arange(QCOLS), aidx.shape[1])
        ll = local.ravel()
        m[ll // 128, ll % 128, qq] = 1.0
        # repack [8,128,512] -> [4,128,1024] (chunk pairs side by side)
        m = m.reshape(4, 2, 128, QCOLS).transpose(0, 2, 1, 3).reshape(
            4, 128, 1024)
        in_maps.append({
            "xs": slab,
            "wqkvT": wqkvT,
            "bqkv": bqkv,
            "wprojT": wprojT,
            "bproj": bproj,
            "mask": np.ascontiguousarray(m).astype(bf),
        })
    return in_maps


def kernel(x, w_qkv, b_qkv, w_proj, b_proj, attn_idx):
    global LAST_EXEC_NS, LAST_TRACE
    _register_ntff_hook()
    if "graph" not in _NC_CACHE:
        _NC_CACHE["graph"] = build_graph()
    nc = _NC_CACHE["graph"]
    in_maps = _build_inputs(x, w_qkv, b_qkv, w_proj, b_proj, attn_idx)
    trace = bool(int(os.environ.get("BASSK_TRACE", "0")))
    res = run_bass_kernel_spmd(nc, in_maps, core_ids=list(range(NCORES)),
                               trace=trace)
    LAST_EXEC_NS = res.exec_time_ns
    if res.instructions_and_trace is not None:
        LAST_TRACE = res.instructions_and_trace[1]
    out = np.empty((B, C, HH, WW), np.float32)
    for i in range(NCORES):
        o = res.results[i]["out"].reshape(B, C, ROWS, WW)
        out[:, :, 8 * i:8 * i + ROWS, :] = o
    return out
